# revision 1
# baseline (speedup 1.0000x reference)
import os
import sys

sys.path.insert(0, "/opt/trn_rl_repo")

import ml_dtypes
import numpy as np

import concourse.bass as bass
from concourse import bacc
import concourse.mybir as mybir
import concourse.tile as tile
from concourse.bass_utils import run_bass_kernel_spmd
from concourse.masks import make_identity

N = 50000
E = 800000
C = 256
NCORES = 8
SHARD = 6272          # padded rows per core (6250 real), multiple of 128
NP = SHARD * NCORES   # 50176 padded total rows
TILES = SHARD // 128  # 49 dest tiles per core
RT = NP + 128         # gather-table rows (128 zero dummy rows at the end)
DUMMY = NP            # dummy gather index -> zero row
EPS = 1e-5
F32 = mybir.dt.float32
I32 = mybir.dt.int32
BF = mybir.dt.bfloat16
LAYER_CINS = (128, 256, 256)
KCAP = 8              # max slabs per wide gather chunk


def _build_nc(S, K):
    """One SPMD Bass program; S = total message slabs, K[t] = slabs of dest tile t."""
    nc = bacc.Bacc(None, target_bir_lowering=False)

    x_me = nc.declare_dram_parameter("x_me", [128, SHARD], BF, isOutput=False)
    t2l0 = nc.declare_dram_parameter("t2l0", [RT, C], BF, isOutput=False)
    idx = nc.declare_dram_parameter("idx", [128, S], I32, isOutput=False)
    Ws = {}
    for li, cin in enumerate(LAYER_CINS):
        for nm, shp in ((f"W1_{li}", [cin, C]), (f"W2_{li}", [cin, C]),
                        (f"g_{li}", [C]), (f"b_{li}", [C])):
            Ws[nm] = nc.declare_dram_parameter(nm, shp, F32, isOutput=False)
    out_ext = nc.declare_dram_parameter("out", [2, 128, SHARD], F32, isOutput=True)

    T2shard = nc.dram_tensor("T2shard", [SHARD, C], BF)          # my H @ W2 rows
    T2full = [nc.dram_tensor(f"T2full{i}", [RT, C], BF, addr_space="Shared") for i in range(2)]
    stat_in = nc.dram_tensor("stat_in", [128, 4], F32)
    stat_out = nc.dram_tensor("stat_out", [128, 4], F32)

    soff = [0]
    for t in range(TILES):
        soff.append(soff[-1] + K[t])

    with tile.TileContext(nc) as tc:
        with (
            tc.tile_pool(name="persist", bufs=1) as pp,
            tc.tile_pool(name="wpool", bufs=1) as wp,
            tc.tile_pool(name="slab", bufs=6) as sp,
            tc.tile_pool(name="tout", bufs=4) as tp,
            tc.tile_pool(name="misc", bufs=2) as mp,
            tc.tile_pool(name="sq", bufs=3) as qp,
            tc.tile_pool(name="psum", bufs=2, space="PSUM") as psp,
            tc.tile_pool(name="psumT", bufs=2, space="PSUM") as pspT,
        ):
            ident = pp.tile([128, 128], F32)
            make_identity(nc, ident[:])
            agg = pp.tile([128, TILES * 256], F32)     # row-major per-tile accum
            o_t = [pp.tile([128, SHARD], BF, name=f"o_t{i}") for i in range(2)]
            hnew_bf = [pp.tile([128, SHARD], BF, name=f"hnbf{i}") for i in range(2)]
            x_sb = pp.tile([128, SHARD], BF)
            idx_sb = pp.tile([128, S], I32)
            nc.sync.dma_start(out=x_sb[:], in_=x_me[:, :])
            nc.sync.dma_start(out=idx_sb[:], in_=idx[:, :])

            stat_s = pp.tile([128, 2 * TILES], F32)   # per-tile chan sums (oc-interleaved)
            stat_q = pp.tile([128, 2 * TILES], F32)   # per-tile chan sum-squares
            sqjunk = mp.tile([128, 128], BF, name="sqjunk")
            epst = pp.tile([128, 1], F32)
            nc.vector.memset(epst[:], EPS)
            # zero the dummy tail of the gather table (stays zero all layers)
            ztile = mp.tile([128, C], BF)
            nc.vector.memset(ztile[:], 0.0)
            nc.sync.dma_start(out=T2full[0][NP:RT, :], in_=ztile[:])
            nc.sync.dma_start(out=T2full[1][NP:RT, :], in_=ztile[:])

            # preload all weights
            w1s, w2s, gbs = [], [], []
            for li, cin in enumerate(LAYER_CINS):
                kcs = cin // 128
                w1 = [wp.tile([128, C], BF, name=f"w1_{li}_{kc}") for kc in range(kcs)]
                w2 = [wp.tile([128, C], BF, name=f"w2_{li}_{kc}") for kc in range(kcs)]
                for kc in range(kcs):
                    wst = mp.tile([128, 2 * C], F32, name="wst")
                    nc.sync.dma_start(out=wst[:, :C], in_=Ws[f"W1_{li}"][kc * 128:(kc + 1) * 128, :])
                    nc.sync.dma_start(out=wst[:, C:], in_=Ws[f"W2_{li}"][kc * 128:(kc + 1) * 128, :])
                    nc.vector.tensor_copy(out=w1[kc][:], in_=wst[:, :C])
                    nc.vector.tensor_copy(out=w2[kc][:], in_=wst[:, C:])
                gb = wp.tile([128, 4], F32, name=f"gb_{li}")  # cols: g0 g1 b0 b1
                for oc in range(2):
                    nc.sync.dma_start(out=gb[:, oc:oc + 1], in_=Ws[f"g_{li}"][oc * 128:(oc + 1) * 128, None])
                    nc.sync.dma_start(out=gb[:, 2 + oc:3 + oc], in_=Ws[f"b_{li}"][oc * 128:(oc + 1) * 128, None])
                w1s.append(w1); w2s.append(w2); gbs.append(gb)

            for li, cin in enumerate(LAYER_CINS):
                kcs = cin // 128
                w1, w2, gb = w1s[li], w2s[li], gbs[li]
                Hpl = [x_sb] if li == 0 else hnew_bf

                # -- A: local GEMMs; agg <- H@W1 (f32); T2shard <- H@W2 (bf16) --
                # (layer 0's table is host-computed: no T2 GEMM / AllGather)
                for t in range(TILES):
                    if li > 0:
                        ps2 = psp.tile([128, C], F32, space="PSUM")
                        for kc in range(kcs):
                            nc.tensor.matmul(out=ps2[:], lhsT=Hpl[kc][:, t * 128:(t + 1) * 128],
                                             rhs=w2[kc][:], start=(kc == 0), stop=(kc == kcs - 1))
                        ob = tp.tile([128, C], BF)
                        nc.scalar.activation(out=ob[:], in_=ps2[:], func=mybir.ActivationFunctionType.Copy)
                        nc.sync.dma_start(out=T2shard[t * 128:(t + 1) * 128, :], in_=ob[:])
                    ps1 = psp.tile([128, C], F32, space="PSUM")
                    for kc in range(kcs):
                        nc.tensor.matmul(out=ps1[:], lhsT=Hpl[kc][:, t * 128:(t + 1) * 128],
                                         rhs=w1[kc][:], start=(kc == 0), stop=(kc == kcs - 1))
                    nc.scalar.activation(out=agg[:, t * 256:(t + 1) * 256], in_=ps1[:],
                                         func=mybir.ActivationFunctionType.Copy)

                if li > 0:
                    # -- AllGather the T2 shard into the full gather table --
                    nc.gpsimd.collective_compute(
                        "AllGather", mybir.AluOpType.bypass,
                        replica_groups=[list(range(NCORES))],
                        ins=[T2shard.ap().opt()], outs=[T2full[li % 2][0:NP, :].opt()])
                tbl = t2l0 if li == 0 else T2full[li % 2]

                # -- B: per dest tile, wide gather + tree-add into agg --
                for t in range(TILES):
                    at = agg[:, t * 256:(t + 1) * 256]
                    s0 = soff[t]
                    for c0 in range(0, K[t], KCAP):
                        k = min(KCAP, K[t] - c0)
                        wide = sp.tile([128, KCAP * 256], BF)
                        for j in range(k):
                            nc.gpsimd.indirect_dma_start(
                                out=wide[:, j * 256:(j + 1) * 256], out_offset=None,
                                in_=tbl[:, :],
                                in_offset=bass.IndirectOffsetOnAxis(
                                    ap=idx_sb[:, s0 + c0 + j:s0 + c0 + j + 1], axis=0))
                        # bf16 pairwise tree within the chunk, then one f32 add
                        m = k
                        while m > 1:
                            h = m // 2
                            nc.vector.tensor_tensor(
                                out=wide[:, :h * 256], in0=wide[:, :h * 256],
                                in1=wide[:, h * 256:2 * h * 256], op=mybir.AluOpType.add)
                            if m % 2:
                                nc.vector.tensor_tensor(
                                    out=wide[:, :C], in0=wide[:, :C],
                                    in1=wide[:, (m - 1) * 256:m * 256], op=mybir.AluOpType.add)
                            m = h
                        nc.vector.tensor_tensor(out=at, in0=at, in1=wide[:, :C],
                                                op=mybir.AluOpType.add)
                    # transpose to channel-major planes + incremental stats
                    for oc in range(2):
                        pt = pspT.tile([128, 128], F32, space="PSUM")
                        nc.tensor.transpose(out=pt[:], in_=agg[:, t * 256 + oc * 128: t * 256 + (oc + 1) * 128],
                                            identity=ident[:])
                        ot_sl = o_t[oc][:, t * 128:(t + 1) * 128]
                        nc.scalar.activation(out=ot_sl, in_=pt[:],
                                             func=mybir.ActivationFunctionType.Copy)
                        nc.vector.reduce_sum(out=stat_s[:, 2 * t + oc:2 * t + oc + 1],
                                             in_=ot_sl, axis=mybir.AxisListType.X)
                        nc.scalar.activation(out=sqjunk[:], in_=ot_sl,
                                             func=mybir.ActivationFunctionType.Square,
                                             accum_out=stat_q[:, 2 * t + oc:2 * t + oc + 1])

                # -- partial BN stats, tiny AllReduce --
                st = mp.tile([128, 4], F32)
                for oc in range(2):
                    nc.vector.reduce_sum(out=st[:, oc:oc + 1],
                                         in_=stat_s[:].rearrange("p (t o) -> p o t", o=2)[:, oc, :],
                                         axis=mybir.AxisListType.X)
                    nc.vector.reduce_sum(out=st[:, 2 + oc:3 + oc],
                                         in_=stat_q[:].rearrange("p (t o) -> p o t", o=2)[:, oc, :],
                                         axis=mybir.AxisListType.X)
                nc.sync.dma_start(out=stat_in[:, :], in_=st[:])
                nc.gpsimd.collective_compute(
                    "AllReduce", mybir.AluOpType.add,
                    replica_groups=[list(range(NCORES))],
                    ins=[stat_in.ap().opt()], outs=[stat_out.ap().opt()])
                gst = mp.tile([128, 4], F32)
                nc.sync.dma_start(out=gst[:], in_=stat_out[:, :])

                sc = mp.tile([128, 8], F32)
                inv_n = 1.0 / N
                for oc in range(2):
                    mean = sc[:, oc:oc + 1]
                    nc.vector.tensor_scalar_mul(out=mean, in0=gst[:, oc:oc + 1], scalar1=inv_n)
                    ex2 = sc[:, 2 + oc:3 + oc]
                    nc.vector.tensor_scalar_mul(out=ex2, in0=gst[:, 2 + oc:3 + oc], scalar1=inv_n)
                    var = sc[:, 4 + oc:5 + oc]
                    nc.vector.tensor_tensor(out=var, in0=mean, in1=mean, op=mybir.AluOpType.mult)
                    nc.vector.tensor_tensor(out=var, in0=ex2, in1=var, op=mybir.AluOpType.subtract)
                    std = sc[:, 6 + oc:7 + oc]
                    nc.scalar.activation(out=std, in_=var, func=mybir.ActivationFunctionType.Sqrt, bias=epst[:])
                    nc.vector.reciprocal(out=std, in_=std)  # now rstd
                    scale = sc[:, 4 + oc:5 + oc]  # overwrite var slot
                    nc.vector.tensor_tensor(out=scale, in0=gb[:, oc:oc + 1], in1=std, op=mybir.AluOpType.mult)
                    tmp = sc[:, oc:oc + 1]  # mean slot -> mean*scale
                    nc.vector.tensor_tensor(out=tmp, in0=mean, in1=scale, op=mybir.AluOpType.mult)
                    shift = sc[:, 6 + oc:7 + oc]  # overwrite rstd slot (already consumed)
                    nc.vector.tensor_tensor(out=shift, in0=gb[:, 2 + oc:3 + oc], in1=tmp,
                                            op=mybir.AluOpType.subtract)

                # -- fused BN+ReLU on my shard --
                last = li == len(LAYER_CINS) - 1
                if last:
                    CH = SHARD // 4
                    for oc in range(2):
                        for c0 in range(0, SHARD, CH):
                            ho = qp.tile([128, CH], F32)
                            nc.scalar.activation(out=ho[:], in_=o_t[oc][:, c0:c0 + CH],
                                                 func=mybir.ActivationFunctionType.Relu,
                                                 scale=sc[:, 4 + oc:5 + oc], bias=sc[:, 6 + oc:7 + oc])
                            nc.sync.dma_start(out=out_ext[oc, :, c0:c0 + CH], in_=ho[:])
                else:
                    for oc in range(2):
                        nc.scalar.activation(out=hnew_bf[oc][:], in_=o_t[oc][:],
                                             func=mybir.ActivationFunctionType.Relu,
                                             scale=sc[:, 4 + oc:5 + oc], bias=sc[:, 6 + oc:7 + oc])
    nc.compile()
    return nc


def kernel(x, edge_index, W1_0, W2_0, g_0, b_0, W1_1, W2_1, g_1, b_1, W1_2, W2_2, g_2, b_2):
    x = np.asarray(x, np.float32)
    ei = np.asarray(edge_index)
    row, col = ei[0].astype(np.int64), ei[1].astype(np.int64)

    deg = np.bincount(row, minlength=N)
    order = np.argsort(-deg, kind="stable")          # new global rank -> old id
    newpos = np.empty(N, np.int64)
    for s in range(NCORES):
        olds = order[s::NCORES]
        newpos[olds] = s * SHARD + np.arange(olds.shape[0])

    rnew, cnew = newpos[row], newpos[col]
    srt = np.argsort(rnew, kind="stable")
    rs, cs = rnew[srt], cnew[srt]
    counts = np.bincount(rnew, minlength=NP)
    cum = np.concatenate([[0], np.cumsum(counts)])
    rank = np.arange(E) - cum[rs]

    tloc = (np.arange(NP) % SHARD) // 128
    K = np.zeros(TILES, np.int64)
    np.maximum.at(K, tloc, counts)
    K = K.astype(int)
    soff = np.concatenate([[0], np.cumsum(K)])
    S = int(soff[-1])

    idx_all = np.full((NCORES, S, 128), DUMMY, np.int32)
    e_s = rs // SHARD
    e_t = (rs % SHARD) // 128
    e_p = (rs % SHARD) % 128
    idx_all[e_s, soff[e_t] + rank, e_p] = cs.astype(np.int32)

    x_me = np.zeros((NCORES, 128, SHARD), ml_dtypes.bfloat16)
    for s in range(NCORES):
        olds = order[s::NCORES]
        x_me[s, :, :olds.shape[0]] = x[olds].T.astype(ml_dtypes.bfloat16)

    # host-computed layer-0 gather table: rows in new-position order
    xg = np.zeros((RT, x.shape[1]), np.float32)
    for s in range(NCORES):
        olds = order[s::NCORES]
        xg[s * SHARD:s * SHARD + olds.shape[0]] = x[olds]
    t2l0 = (xg @ np.asarray(W2_0, np.float32)).astype(ml_dtypes.bfloat16)

    weights = {"W1_0": W1_0, "W2_0": W2_0, "g_0": g_0, "b_0": b_0,
               "W1_1": W1_1, "W2_1": W2_1, "g_1": g_1, "b_1": b_1,
               "W1_2": W1_2, "W2_2": W2_2, "g_2": g_2, "b_2": b_2}
    weights = {k: np.ascontiguousarray(np.asarray(v, np.float32)) for k, v in weights.items()}

    nc = _build_nc(S, list(K))
    in_maps = []
    for s in range(NCORES):
        m = {"x_me": np.ascontiguousarray(x_me[s]),
             "t2l0": t2l0,
             "idx": np.ascontiguousarray(idx_all[s].T)}
        m.update(weights)
        in_maps.append(m)

    _tr = bool(os.environ.get("BASS_TRACE_RUN"))
    _td = os.environ.get("BASS_TRACE_DIR") or None
    if _td:
        os.makedirs(_td, exist_ok=True)
    r = run_bass_kernel_spmd(nc, in_maps, list(range(NCORES)), trace=_tr, tmpdir=_td)
    if _tr:
        print("HW exec time:", r.exec_time_ns, "ns", flush=True)
        if r.profile_json:
            print("profile_json:", r.profile_json, flush=True)
    res = r.results

    out = np.empty((N, C), np.float32)
    for s in range(NCORES):
        o = res[s]["out"] if isinstance(res[s], dict) else res[s][0]
        o = np.asarray(o).reshape(2, 128, SHARD)
        blk = np.transpose(o, (2, 0, 1)).reshape(SHARD, C)
        out[order[s::NCORES]] = blk[:6250]
    return out



# revision 6
# speedup vs baseline: 1.0310x; 1.0310x over previous
import os
import sys

sys.path.insert(0, "/opt/trn_rl_repo")

import ml_dtypes
import numpy as np

import concourse.bass as bass
from concourse import bacc
import concourse.mybir as mybir
import concourse.tile as tile
from concourse.bass_utils import run_bass_kernel_spmd
from concourse.masks import make_identity

N = 50000
E = 800000
C = 256
NCORES = 8
SHARD = 6272          # padded rows per core (6250 real), multiple of 128
NP = SHARD * NCORES   # 50176 padded total rows
TILES = SHARD // 128  # 49 dest tiles per core
EPS = 1e-5
F32 = mybir.dt.float32
I16 = mybir.dt.int16
BF = mybir.dt.bfloat16
LAYER_CINS = (128, 256, 256)

# split-AllGather table layout: region A = first ROW_A rows of each shard,
# region B = the rest.  Each region is a separate DRAM tensor whose row count
# stays below the int16 index limit of dma_gather (32767).
ROW_A = 3200                       # 25 dest tiles
ROW_B = SHARD - ROW_A              # 3072 = 24 tiles
TILES_A = ROW_A // 128
WA_ROWS = NCORES * ROW_A           # 25600
WB_ROWS = NCORES * ROW_B           # 24576
GROUP_CAP = 48                     # max slabs buffered per (group, window)
DMA_SCRATCH = 16384                # SWDGE descriptor carveout (bytes/partition)
CALL_SLABS = 8                     # max slabs per dma_gather call (64 descs/engine ring cap)


def _build_nc(slabs, groups, tsoff, S):
    """slabs[t][w] slab counts; groups = [(tiles, (stA,nsA), (stB,nsB))];
    tsoff[t][w] = global slab offset of tile t window w; S = total slabs."""
    nc = bacc.Bacc(None, target_bir_lowering=False,
                   dynamic_dma_scratch_size=DMA_SCRATCH)

    x_me = nc.declare_dram_parameter("x_me", [128, SHARD], BF, isOutput=False)
    t2l0A = nc.declare_dram_parameter("t2l0A", [WA_ROWS, C], BF, isOutput=False)
    t2l0B = nc.declare_dram_parameter("t2l0B", [WB_ROWS, C], BF, isOutput=False)
    idx = nc.declare_dram_parameter("idx", [128, S * 8], I16, isOutput=False)
    dvec = nc.declare_dram_parameter("dvec", [128, S], F32, isOutput=False)
    Ws = {}
    for li, cin in enumerate(LAYER_CINS):
        for nm, shp in ((f"W1_{li}", [cin, C]), (f"W2_{li}", [cin, C]),
                        (f"g_{li}", [C]), (f"b_{li}", [C])):
            Ws[nm] = nc.declare_dram_parameter(nm, shp, F32, isOutput=False)
    out_ext = nc.declare_dram_parameter("out", [2, 128, SHARD], F32, isOutput=True)

    T2shardA = nc.dram_tensor("T2shardA", [ROW_A, C], BF)
    T2shardB = nc.dram_tensor("T2shardB", [ROW_B, C], BF)
    T2A = [nc.dram_tensor(f"T2A{i}", [WA_ROWS, C], BF, addr_space="Shared") for i in range(2)]
    T2B = [nc.dram_tensor(f"T2B{i}", [WB_ROWS, C], BF, addr_space="Shared") for i in range(2)]
    stat_in = nc.dram_tensor("stat_in", [128, 4], F32)
    stat_out = nc.dram_tensor("stat_out", [128, 4], F32)

    with tile.TileContext(nc) as tc:
        with (
            tc.tile_pool(name="persist", bufs=1) as pp,
            tc.tile_pool(name="wpool", bufs=1) as wp,
            tc.tile_pool(name="slab", bufs=4) as sp,
            tc.tile_pool(name="gpool", bufs=8) as gp,
            tc.tile_pool(name="tout", bufs=4) as tp,
            tc.tile_pool(name="misc", bufs=2) as mp,
            tc.tile_pool(name="psum", bufs=2, space="PSUM") as psp,
            tc.tile_pool(name="psumT", bufs=2, space="PSUM") as pspT,
        ):
            ident = pp.tile([128, 128], BF)
            make_identity(nc, ident[:])
            iota = pp.tile([128, 128], F32)
            nc.gpsimd.iota(iota[:], [[1, 128]], channel_multiplier=0,
                           allow_small_or_imprecise_dtypes=True)
            o_t = [pp.tile([128, SHARD], BF, name=f"o_t{i}") for i in range(2)]
            hnew_bf = [pp.tile([128, SHARD], BF, name=f"hnbf{i}") for i in range(2)]
            x_sb = pp.tile([128, SHARD], BF)
            idx_sb = pp.tile([128, S * 8], I16)
            dvec_sb = pp.tile([128, S], F32)
            nc.sync.dma_start(out=x_sb[:], in_=x_me[:, :])
            nc.sync.dma_start(out=idx_sb[:], in_=idx[:, :])
            nc.sync.dma_start(out=dvec_sb[:], in_=dvec[:, :])

            stat_s = pp.tile([128, 2 * TILES], F32)   # per-tile chan sums
            stat_q = pp.tile([128, 2 * TILES], F32)   # per-tile chan sum-squares
            sqjunk = mp.tile([128, 128], BF, name="sqjunk")
            epst = pp.tile([128, 1], F32)
            nc.vector.memset(epst[:], EPS)

            # preload all weights
            w1s, w2s, gbs = [], [], []
            for li, cin in enumerate(LAYER_CINS):
                kcs = cin // 128
                w1 = [wp.tile([128, C], BF, name=f"w1_{li}_{kc}") for kc in range(kcs)]
                w2 = [wp.tile([128, C], BF, name=f"w2_{li}_{kc}") for kc in range(kcs)]
                for kc in range(kcs):
                    wst = mp.tile([128, 2 * C], F32, name="wst")
                    nc.sync.dma_start(out=wst[:, :C], in_=Ws[f"W1_{li}"][kc * 128:(kc + 1) * 128, :])
                    nc.sync.dma_start(out=wst[:, C:], in_=Ws[f"W2_{li}"][kc * 128:(kc + 1) * 128, :])
                    nc.vector.tensor_copy(out=w1[kc][:], in_=wst[:, :C])
                    nc.vector.tensor_copy(out=w2[kc][:], in_=wst[:, C:])
                gb = wp.tile([128, 4], F32, name=f"gb_{li}")  # cols: g0 g1 b0 b1
                for oc in range(2):
                    nc.sync.dma_start(out=gb[:, oc:oc + 1], in_=Ws[f"g_{li}"][oc * 128:(oc + 1) * 128, None])
                    nc.sync.dma_start(out=gb[:, 2 + oc:3 + oc], in_=Ws[f"b_{li}"][oc * 128:(oc + 1) * 128, None])
                w1s.append(w1); w2s.append(w2); gbs.append(gb)

            for li, cin in enumerate(LAYER_CINS):
                kcs = cin // 128
                w1, w2, gb = w1s[li], w2s[li], gbs[li]
                Hpl = [x_sb] if li == 0 else hnew_bf

                # -- T2 tables: local H @ W2 shards + split AllGather --
                # (layer 0's tables are host-computed)
                if li > 0:
                    for t in range(TILES_A):
                        ps2 = psp.tile([128, C], F32, space="PSUM")
                        for kc in range(kcs):
                            nc.tensor.matmul(out=ps2[:], lhsT=Hpl[kc][:, t * 128:(t + 1) * 128],
                                             rhs=w2[kc][:], start=(kc == 0), stop=(kc == kcs - 1))
                        ob = tp.tile([128, C], BF)
                        nc.scalar.activation(out=ob[:], in_=ps2[:], func=mybir.ActivationFunctionType.Copy)
                        nc.sync.dma_start(out=T2shardA[t * 128:(t + 1) * 128, :], in_=ob[:])
                    nc.gpsimd.collective_compute(
                        "AllGather", mybir.AluOpType.bypass,
                        replica_groups=[list(range(NCORES))],
                        ins=[T2shardA.ap().opt()], outs=[T2A[li % 2].ap().opt()])
                    for t in range(TILES_A, TILES):
                        ps2 = psp.tile([128, C], F32, space="PSUM")
                        for kc in range(kcs):
                            nc.tensor.matmul(out=ps2[:], lhsT=Hpl[kc][:, t * 128:(t + 1) * 128],
                                             rhs=w2[kc][:], start=(kc == 0), stop=(kc == kcs - 1))
                        ob = tp.tile([128, C], BF)
                        nc.scalar.activation(out=ob[:], in_=ps2[:], func=mybir.ActivationFunctionType.Copy)
                        nc.sync.dma_start(out=T2shardB[(t - TILES_A) * 128:(t - TILES_A + 1) * 128, :], in_=ob[:])
                    nc.gpsimd.collective_compute(
                        "AllGather", mybir.AluOpType.bypass,
                        replica_groups=[list(range(NCORES))],
                        ins=[T2shardB.ap().opt()], outs=[T2B[li % 2].ap().opt()])
                tbl = (t2l0A, t2l0B) if li == 0 else (T2A[li % 2], T2B[li % 2])

                # -- gather + matmul-scatter aggregation per group --
                for tl, spans in groups:
                    wides = {}
                    for w in (0, 1):
                        st, ns = spans[w]
                        if ns == 0:
                            continue
                        wt = sp.tile([128, ns * 256], BF)
                        for c0 in range(0, ns, CALL_SLABS):
                            cn = min(CALL_SLABS, ns - c0)
                            nc.gpsimd.dma_gather(
                                out_ap=wt[:, c0 * 256:(c0 + cn) * 256].rearrange(
                                    "p (s e) -> p s e", e=256),
                                in_ap=tbl[w][:, :],
                                idxs_ap=idx_sb[:, (st + c0) * 8:(st + c0 + cn) * 8],
                                num_idxs=cn * 128,
                                num_idxs_reg=cn * 128,
                                elem_size=256)
                        wides[w] = (wt, st)
                    for t in tl:
                        ps = psp.tile([128, C], F32, space="PSUM")
                        for kc in range(kcs):
                            nc.tensor.matmul(out=ps[:], lhsT=Hpl[kc][:, t * 128:(t + 1) * 128],
                                             rhs=w1[kc][:], start=(kc == 0), stop=False)
                        nslab_t = slabs[t][0] + slabs[t][1]
                        k = 0
                        for w in (0, 1):
                            if slabs[t][w] == 0:
                                continue
                            wt, gst = wides[w]
                            j0 = tsoff[t][w] - gst
                            for jj in range(slabs[t][w]):
                                gslab = tsoff[t][w] + jj
                                j = j0 + jj
                                G = gp.tile([128, 128], BF)
                                nc.vector.tensor_scalar(
                                    out=G[:], in0=iota[:],
                                    scalar1=dvec_sb[:, gslab:gslab + 1], scalar2=None,
                                    op0=mybir.AluOpType.is_equal)
                                k += 1
                                nc.tensor.matmul(out=ps[:], lhsT=G[:],
                                                 rhs=wt[:, j * 256:(j + 1) * 256],
                                                 start=False, stop=(k == nslab_t))
                        # transpose to channel-major planes + incremental stats
                        rt = tp.tile([128, C], BF)
                        nc.scalar.activation(out=rt[:], in_=ps[:],
                                             func=mybir.ActivationFunctionType.Copy)
                        for oc in range(2):
                            pt = pspT.tile([128, 128], BF, space="PSUM")
                            nc.tensor.transpose(out=pt[:], in_=rt[:, oc * 128:(oc + 1) * 128],
                                                identity=ident[:])
                            ot_sl = o_t[oc][:, t * 128:(t + 1) * 128]
                            nc.scalar.activation(out=ot_sl, in_=pt[:],
                                                 func=mybir.ActivationFunctionType.Copy)
                            nc.vector.reduce_sum(out=stat_s[:, 2 * t + oc:2 * t + oc + 1],
                                                 in_=ot_sl, axis=mybir.AxisListType.X)
                            nc.scalar.activation(out=sqjunk[:], in_=ot_sl,
                                                 func=mybir.ActivationFunctionType.Square,
                                                 accum_out=stat_q[:, 2 * t + oc:2 * t + oc + 1])

                # -- partial BN stats, tiny AllReduce --
                st = mp.tile([128, 4], F32)
                for oc in range(2):
                    nc.vector.reduce_sum(out=st[:, oc:oc + 1],
                                         in_=stat_s[:].rearrange("p (t o) -> p o t", o=2)[:, oc, :],
                                         axis=mybir.AxisListType.X)
                    nc.vector.reduce_sum(out=st[:, 2 + oc:3 + oc],
                                         in_=stat_q[:].rearrange("p (t o) -> p o t", o=2)[:, oc, :],
                                         axis=mybir.AxisListType.X)
                nc.sync.dma_start(out=stat_in[:, :], in_=st[:])
                nc.gpsimd.collective_compute(
                    "AllReduce", mybir.AluOpType.add,
                    replica_groups=[list(range(NCORES))],
                    ins=[stat_in.ap().opt()], outs=[stat_out.ap().opt()])
                gst = mp.tile([128, 4], F32)
                nc.sync.dma_start(out=gst[:], in_=stat_out[:, :])

                sc = mp.tile([128, 8], F32)
                inv_n = 1.0 / N
                for oc in range(2):
                    mean = sc[:, oc:oc + 1]
                    nc.vector.tensor_scalar_mul(out=mean, in0=gst[:, oc:oc + 1], scalar1=inv_n)
                    ex2 = sc[:, 2 + oc:3 + oc]
                    nc.vector.tensor_scalar_mul(out=ex2, in0=gst[:, 2 + oc:3 + oc], scalar1=inv_n)
                    var = sc[:, 4 + oc:5 + oc]
                    nc.vector.tensor_tensor(out=var, in0=mean, in1=mean, op=mybir.AluOpType.mult)
                    nc.vector.tensor_tensor(out=var, in0=ex2, in1=var, op=mybir.AluOpType.subtract)
                    std = sc[:, 6 + oc:7 + oc]
                    nc.scalar.activation(out=std, in_=var, func=mybir.ActivationFunctionType.Sqrt, bias=epst[:])
                    nc.vector.reciprocal(out=std, in_=std)  # now rstd
                    scale = sc[:, 4 + oc:5 + oc]  # overwrite var slot
                    nc.vector.tensor_tensor(out=scale, in0=gb[:, oc:oc + 1], in1=std, op=mybir.AluOpType.mult)
                    tmp = sc[:, oc:oc + 1]  # mean slot -> mean*scale
                    nc.vector.tensor_tensor(out=tmp, in0=mean, in1=scale, op=mybir.AluOpType.mult)
                    shift = sc[:, 6 + oc:7 + oc]  # overwrite rstd slot (already consumed)
                    nc.vector.tensor_tensor(out=shift, in0=gb[:, 2 + oc:3 + oc], in1=tmp,
                                            op=mybir.AluOpType.subtract)

                # -- fused BN+ReLU on my shard --
                last = li == len(LAYER_CINS) - 1
                if last:
                    CH = SHARD // 4
                    for oc in range(2):
                        for c0 in range(0, SHARD, CH):
                            ho = mp.tile([128, CH], F32, name="ho")
                            nc.scalar.activation(out=ho[:], in_=o_t[oc][:, c0:c0 + CH],
                                                 func=mybir.ActivationFunctionType.Relu,
                                                 scale=sc[:, 4 + oc:5 + oc], bias=sc[:, 6 + oc:7 + oc])
                            nc.sync.dma_start(out=out_ext[oc, :, c0:c0 + CH], in_=ho[:])
                else:
                    for oc in range(2):
                        nc.scalar.activation(out=hnew_bf[oc][:], in_=o_t[oc][:],
                                             func=mybir.ActivationFunctionType.Relu,
                                             scale=sc[:, 4 + oc:5 + oc], bias=sc[:, 6 + oc:7 + oc])
    nc.compile()
    return nc


def _host_prep(x, edge_index, W2_0):
    row, col = edge_index[0].astype(np.int64), edge_index[1].astype(np.int64)

    deg = np.bincount(row, minlength=N)
    order = np.argsort(-deg, kind="stable")          # new global rank -> old id
    newpos = np.empty(N, np.int64)
    for s in range(NCORES):
        olds = order[s::NCORES]
        newpos[olds] = s * SHARD + np.arange(olds.shape[0])

    rnew, cnew = newpos[row], newpos[col]

    # table row of each source under split-AllGather layout, and its window
    s_src, o_src = cnew // SHARD, cnew % SHARD
    in_a = o_src < ROW_A
    trow = np.where(in_a, s_src * ROW_A + o_src,
                    s_src * ROW_B + (o_src - ROW_A))   # already window-local
    win = (~in_a).astype(np.int64)

    # shared slab structure: per (tile, window) max count over cores
    core = rnew // SHARD
    t_loc = (rnew % SHARD) // 128
    p_lane = (rnew % SHARD) % 128
    cnt = np.zeros((NCORES, TILES, 2), np.int64)
    np.add.at(cnt, (core, t_loc, win), 1)
    cmax = cnt.max(axis=0)
    slabs = np.ceil(cmax / 128).astype(np.int64)      # [TILES, 2]

    # adaptive groups with per-window slab cap
    groups_t = []
    cur, a, b = [], 0, 0
    for t in range(TILES):
        w0, w1 = int(slabs[t][0]), int(slabs[t][1])
        if cur and max(a + w0, b + w1) > GROUP_CAP:
            groups_t.append(cur); cur, a, b = [], 0, 0
        cur.append(t); a += w0; b += w1
    groups_t.append(cur)

    # global slab order: per group, window-A slabs of its tiles then window-B
    tsoff = np.zeros((TILES, 2), np.int64)
    groups = []
    off = 0
    for tl in groups_t:
        spans = []
        for w in (0, 1):
            st = off
            for t in tl:
                tsoff[t][w] = off
                off += int(slabs[t][w])
            spans.append((st, off - st))
        groups.append((tl, spans))
    S = off

    # per-core idx / dvec tables
    idx_all = np.zeros((NCORES, 128, S * 8), np.int16)
    dvec_all = np.full((NCORES, 128, S), -1.0, np.float32)
    slot_base = np.zeros((TILES, 2), np.int64)
    slot_base[:, :] = tsoff * 128
    for s in range(NCORES):
        m = core == s
        et, ew, eloc, ep = t_loc[m], win[m], trow[m], p_lane[m]
        key = et * 2 + ew
        srt = np.argsort(key, kind="stable")
        key_s = key[srt]
        kcnt = np.bincount(key_s, minlength=TILES * 2)
        kbase = np.concatenate([[0], np.cumsum(kcnt)])[:-1]
        rank = np.arange(key_s.shape[0]) - kbase[key_s]
        slot = slot_base.reshape(-1)[key_s] + rank
        slab_j, lane = slot // 128, slot % 128
        # idx: lane l of slab j -> (partition l%16 in all 8 groups, col 8j+l//16)
        cols = 8 * slab_j + lane // 16
        prow = (lane % 16).astype(np.int64)
        v16 = eloc[srt].astype(np.int16)
        for g in range(8):
            idx_all[s, g * 16 + prow, cols] = v16
        dvec_all[s, lane, slab_j] = ep[srt].astype(np.float32)

    # layer-0 tables: x @ W2_0 in table order
    xg = np.zeros((NP, x.shape[1]), np.float32)
    for s in range(NCORES):
        olds = order[s::NCORES]
        xg[s * SHARD:s * SHARD + olds.shape[0]] = x[olds]
    t2 = (xg @ np.asarray(W2_0, np.float32)).astype(ml_dtypes.bfloat16)
    t2 = t2.reshape(NCORES, SHARD, C)
    t2A = np.ascontiguousarray(t2[:, :ROW_A, :].reshape(WA_ROWS, C))
    t2B = np.ascontiguousarray(t2[:, ROW_A:, :].reshape(WB_ROWS, C))

    x_me = np.zeros((NCORES, 128, SHARD), ml_dtypes.bfloat16)
    for s in range(NCORES):
        olds = order[s::NCORES]
        x_me[s, :, :olds.shape[0]] = x[olds].T.astype(ml_dtypes.bfloat16)

    return (order, slabs, groups, tsoff, S, idx_all, dvec_all, t2A, t2B, x_me)


def kernel(x, edge_index, W1_0, W2_0, g_0, b_0, W1_1, W2_1, g_1, b_1, W1_2, W2_2, g_2, b_2):
    x = np.asarray(x, np.float32)
    ei = np.asarray(edge_index)

    (order, slabs, groups, tsoff, S, idx_all, dvec_all, t2A, t2B, x_me) = \
        _host_prep(x, ei, W2_0)

    weights = {"W1_0": W1_0, "W2_0": W2_0, "g_0": g_0, "b_0": b_0,
               "W1_1": W1_1, "W2_1": W2_1, "g_1": g_1, "b_1": b_1,
               "W1_2": W1_2, "W2_2": W2_2, "g_2": g_2, "b_2": b_2}
    weights = {k: np.ascontiguousarray(np.asarray(v, np.float32)) for k, v in weights.items()}

    nc = _build_nc([list(map(int, s)) for s in slabs], groups,
                   [list(map(int, s)) for s in tsoff], S)
    in_maps = []
    for s in range(NCORES):
        m = {"x_me": np.ascontiguousarray(x_me[s]),
             "t2l0A": t2A, "t2l0B": t2B,
             "idx": np.ascontiguousarray(idx_all[s]),
             "dvec": np.ascontiguousarray(dvec_all[s])}
        m.update(weights)
        in_maps.append(m)

    _tr = bool(os.environ.get("BASS_TRACE_RUN"))
    _td = os.environ.get("BASS_TRACE_DIR") or None
    if _td:
        os.makedirs(_td, exist_ok=True)
    r = run_bass_kernel_spmd(nc, in_maps, list(range(NCORES)), trace=_tr, tmpdir=_td)
    if _tr:
        print("HW exec time:", r.exec_time_ns, "ns", flush=True)
        if r.profile_json:
            print("profile_json:", r.profile_json, flush=True)
    res = r.results

    out = np.empty((N, C), np.float32)
    for s in range(NCORES):
        o = res[s]["out"] if isinstance(res[s], dict) else res[s][0]
        o = np.asarray(o).reshape(2, 128, SHARD)
        blk = np.transpose(o, (2, 0, 1)).reshape(SHARD, C)
        out[order[s::NCORES]] = blk[:6250]
    return out


# revision 7
# speedup vs baseline: 2.2604x; 2.1924x over previous
import os
import sys

sys.path.insert(0, "/opt/trn_rl_repo")

import ml_dtypes
import numpy as np

import concourse.bass as bass
from concourse import bacc
import concourse.mybir as mybir
import concourse.tile as tile
from concourse.bass_utils import run_bass_kernel_spmd
from concourse.masks import make_identity

N = 50000
E = 800000
C = 256
NCORES = 8
SHARD = 6272          # padded rows per core (6250 real), multiple of 128
NP = SHARD * NCORES   # 50176 padded total rows
TILES = SHARD // 128  # 49 dest tiles per core
EPS = 1e-5
F32 = mybir.dt.float32
I16 = mybir.dt.int16
BF = mybir.dt.bfloat16
LAYER_CINS = (128, 256, 256)

# split-AllGather table layout: region A = first ROW_A rows of each shard,
# region B = the rest.  Each region is a separate DRAM tensor whose row count
# stays below the int16 index limit of dma_gather (32767).
ROW_A = 3200                       # 25 dest tiles
ROW_B = SHARD - ROW_A              # 3072 = 24 tiles
TILES_A = ROW_A // 128
WA_ROWS = NCORES * ROW_A           # 25600
WB_ROWS = NCORES * ROW_B           # 24576
GROUP_CAP = 48                     # max slabs buffered per (group, window)
DMA_SCRATCH = 16384                # SWDGE descriptor carveout (bytes/partition)
CALL_SLABS = 8                     # max slabs per dma_gather call (64 descs/engine ring cap)


def _build_nc(slabs, groups, tsoff, S):
    """slabs[t][w] slab counts; groups = [(tiles, (stA,nsA), (stB,nsB))];
    tsoff[t][w] = global slab offset of tile t window w; S = total slabs."""
    nc = bacc.Bacc(None, target_bir_lowering=False,
                   dynamic_dma_scratch_size=DMA_SCRATCH, num_swdge_queues=4)

    x_me = nc.declare_dram_parameter("x_me", [128, SHARD], BF, isOutput=False)
    t2l0A = nc.declare_dram_parameter("t2l0A", [WA_ROWS, C], BF, isOutput=False)
    t2l0B = nc.declare_dram_parameter("t2l0B", [WB_ROWS, C], BF, isOutput=False)
    idx = nc.declare_dram_parameter("idx", [128, S * 8], I16, isOutput=False)
    dvec = nc.declare_dram_parameter("dvec", [128, S], F32, isOutput=False)
    Ws = {}
    for li, cin in enumerate(LAYER_CINS):
        for nm, shp in ((f"W1_{li}", [cin, C]), (f"W2_{li}", [cin, C]),
                        (f"g_{li}", [C]), (f"b_{li}", [C])):
            Ws[nm] = nc.declare_dram_parameter(nm, shp, F32, isOutput=False)
    out_ext = nc.declare_dram_parameter("out", [2, 128, SHARD], F32, isOutput=True)

    T2shardA = nc.dram_tensor("T2shardA", [ROW_A, C], BF)
    T2shardB = nc.dram_tensor("T2shardB", [ROW_B, C], BF)
    T2A = [nc.dram_tensor(f"T2A{i}", [WA_ROWS, C], BF, addr_space="Shared") for i in range(2)]
    T2B = [nc.dram_tensor(f"T2B{i}", [WB_ROWS, C], BF, addr_space="Shared") for i in range(2)]
    stat_in = nc.dram_tensor("stat_in", [128, 4], F32)
    stat_out = nc.dram_tensor("stat_out", [128, 4], F32)

    with tile.TileContext(nc) as tc:
        with (
            tc.tile_pool(name="persist", bufs=1) as pp,
            tc.tile_pool(name="wpool", bufs=1) as wp,
            tc.tile_pool(name="slab", bufs=4) as sp,
            tc.tile_pool(name="gpool", bufs=8) as gp,
            tc.tile_pool(name="tout", bufs=4) as tp,
            tc.tile_pool(name="misc", bufs=2) as mp,
            tc.tile_pool(name="psum", bufs=2, space="PSUM") as psp,
            tc.tile_pool(name="psumT", bufs=2, space="PSUM") as pspT,
        ):
            ident = pp.tile([128, 128], BF)
            make_identity(nc, ident[:])
            iota = pp.tile([128, 128], F32)
            nc.gpsimd.iota(iota[:], [[1, 128]], channel_multiplier=0,
                           allow_small_or_imprecise_dtypes=True)
            o_t = [pp.tile([128, SHARD], BF, name=f"o_t{i}") for i in range(2)]
            hnew_bf = [pp.tile([128, SHARD], BF, name=f"hnbf{i}") for i in range(2)]
            x_sb = pp.tile([128, SHARD], BF)
            idx_sb = pp.tile([128, S * 8], I16)
            dvec_sb = pp.tile([128, S], F32)
            nc.sync.dma_start(out=x_sb[:], in_=x_me[:, :])
            nc.sync.dma_start(out=idx_sb[:], in_=idx[:, :])
            nc.sync.dma_start(out=dvec_sb[:], in_=dvec[:, :])

            stat_s = pp.tile([128, 2 * TILES], F32)   # per-tile chan sums
            stat_q = pp.tile([128, 2 * TILES], F32)   # per-tile chan sum-squares
            sqjunk = mp.tile([128, 128], BF, name="sqjunk")
            epst = pp.tile([128, 1], F32)
            nc.vector.memset(epst[:], EPS)

            # preload all weights
            w1s, w2s, gbs = [], [], []
            for li, cin in enumerate(LAYER_CINS):
                kcs = cin // 128
                w1 = [wp.tile([128, C], BF, name=f"w1_{li}_{kc}") for kc in range(kcs)]
                w2 = [wp.tile([128, C], BF, name=f"w2_{li}_{kc}") for kc in range(kcs)]
                for kc in range(kcs):
                    wst = mp.tile([128, 2 * C], F32, name="wst")
                    nc.sync.dma_start(out=wst[:, :C], in_=Ws[f"W1_{li}"][kc * 128:(kc + 1) * 128, :])
                    nc.sync.dma_start(out=wst[:, C:], in_=Ws[f"W2_{li}"][kc * 128:(kc + 1) * 128, :])
                    nc.vector.tensor_copy(out=w1[kc][:], in_=wst[:, :C])
                    nc.vector.tensor_copy(out=w2[kc][:], in_=wst[:, C:])
                gb = wp.tile([128, 4], F32, name=f"gb_{li}")  # cols: g0 g1 b0 b1
                for oc in range(2):
                    nc.sync.dma_start(out=gb[:, oc:oc + 1], in_=Ws[f"g_{li}"][oc * 128:(oc + 1) * 128, None])
                    nc.sync.dma_start(out=gb[:, 2 + oc:3 + oc], in_=Ws[f"b_{li}"][oc * 128:(oc + 1) * 128, None])
                w1s.append(w1); w2s.append(w2); gbs.append(gb)

            for li, cin in enumerate(LAYER_CINS):
                kcs = cin // 128
                w1, w2, gb = w1s[li], w2s[li], gbs[li]
                Hpl = [x_sb] if li == 0 else hnew_bf

                # -- T2 tables: local H @ W2 shards + split AllGather --
                # (layer 0's tables are host-computed)
                if li > 0:
                    for t in range(TILES_A):
                        ps2 = psp.tile([128, C], F32, space="PSUM")
                        for kc in range(kcs):
                            nc.tensor.matmul(out=ps2[:], lhsT=Hpl[kc][:, t * 128:(t + 1) * 128],
                                             rhs=w2[kc][:], start=(kc == 0), stop=(kc == kcs - 1))
                        ob = tp.tile([128, C], BF)
                        nc.scalar.activation(out=ob[:], in_=ps2[:], func=mybir.ActivationFunctionType.Copy)
                        nc.sync.dma_start(out=T2shardA[t * 128:(t + 1) * 128, :], in_=ob[:])
                    nc.gpsimd.collective_compute(
                        "AllGather", mybir.AluOpType.bypass,
                        replica_groups=[list(range(NCORES))],
                        ins=[T2shardA.ap().opt()], outs=[T2A[li % 2].ap().opt()])
                    for t in range(TILES_A, TILES):
                        ps2 = psp.tile([128, C], F32, space="PSUM")
                        for kc in range(kcs):
                            nc.tensor.matmul(out=ps2[:], lhsT=Hpl[kc][:, t * 128:(t + 1) * 128],
                                             rhs=w2[kc][:], start=(kc == 0), stop=(kc == kcs - 1))
                        ob = tp.tile([128, C], BF)
                        nc.scalar.activation(out=ob[:], in_=ps2[:], func=mybir.ActivationFunctionType.Copy)
                        nc.sync.dma_start(out=T2shardB[(t - TILES_A) * 128:(t - TILES_A + 1) * 128, :], in_=ob[:])
                    nc.gpsimd.collective_compute(
                        "AllGather", mybir.AluOpType.bypass,
                        replica_groups=[list(range(NCORES))],
                        ins=[T2shardB.ap().opt()], outs=[T2B[li % 2].ap().opt()])
                tbl = (t2l0A, t2l0B) if li == 0 else (T2A[li % 2], T2B[li % 2])

                # -- gather + matmul-scatter aggregation per group --
                qn = 0
                for tl, spans in groups:
                    wides = {}
                    for w in (0, 1):
                        st, ns = spans[w]
                        if ns == 0:
                            continue
                        wt = sp.tile([128, ns * 256], BF)
                        for c0 in range(0, ns, CALL_SLABS):
                            cn = min(CALL_SLABS, ns - c0)
                            nc.gpsimd.dma_gather(
                                out_ap=wt[:, c0 * 256:(c0 + cn) * 256].rearrange(
                                    "p (s e) -> p s e", e=256),
                                in_ap=tbl[w][:, :],
                                idxs_ap=idx_sb[:, (st + c0) * 8:(st + c0 + cn) * 8],
                                num_idxs=cn * 128,
                                num_idxs_reg=cn * 128,
                                elem_size=256,
                                queue_num=qn)
                            qn = (qn + 1) % 4
                        wides[w] = (wt, st)
                    for t in tl:
                        ps = psp.tile([128, C], F32, space="PSUM")
                        for kc in range(kcs):
                            nc.tensor.matmul(out=ps[:], lhsT=Hpl[kc][:, t * 128:(t + 1) * 128],
                                             rhs=w1[kc][:], start=(kc == 0), stop=False)
                        nslab_t = slabs[t][0] + slabs[t][1]
                        k = 0
                        for w in (0, 1):
                            if slabs[t][w] == 0:
                                continue
                            wt, gst = wides[w]
                            j0 = tsoff[t][w] - gst
                            for jj in range(slabs[t][w]):
                                gslab = tsoff[t][w] + jj
                                j = j0 + jj
                                G = gp.tile([128, 128], BF)
                                nc.vector.tensor_scalar(
                                    out=G[:], in0=iota[:],
                                    scalar1=dvec_sb[:, gslab:gslab + 1], scalar2=None,
                                    op0=mybir.AluOpType.is_equal)
                                k += 1
                                nc.tensor.matmul(out=ps[:], lhsT=G[:],
                                                 rhs=wt[:, j * 256:(j + 1) * 256],
                                                 start=False, stop=(k == nslab_t))
                        # transpose to channel-major planes + incremental stats
                        rt = tp.tile([128, C], BF)
                        nc.scalar.activation(out=rt[:], in_=ps[:],
                                             func=mybir.ActivationFunctionType.Copy)
                        for oc in range(2):
                            pt = pspT.tile([128, 128], BF, space="PSUM")
                            nc.tensor.transpose(out=pt[:], in_=rt[:, oc * 128:(oc + 1) * 128],
                                                identity=ident[:])
                            ot_sl = o_t[oc][:, t * 128:(t + 1) * 128]
                            nc.scalar.activation(out=ot_sl, in_=pt[:],
                                                 func=mybir.ActivationFunctionType.Copy)
                            nc.vector.reduce_sum(out=stat_s[:, 2 * t + oc:2 * t + oc + 1],
                                                 in_=ot_sl, axis=mybir.AxisListType.X)
                            nc.scalar.activation(out=sqjunk[:], in_=ot_sl,
                                                 func=mybir.ActivationFunctionType.Square,
                                                 accum_out=stat_q[:, 2 * t + oc:2 * t + oc + 1])

                # -- partial BN stats, tiny AllReduce --
                st = mp.tile([128, 4], F32)
                for oc in range(2):
                    nc.vector.reduce_sum(out=st[:, oc:oc + 1],
                                         in_=stat_s[:].rearrange("p (t o) -> p o t", o=2)[:, oc, :],
                                         axis=mybir.AxisListType.X)
                    nc.vector.reduce_sum(out=st[:, 2 + oc:3 + oc],
                                         in_=stat_q[:].rearrange("p (t o) -> p o t", o=2)[:, oc, :],
                                         axis=mybir.AxisListType.X)
                nc.sync.dma_start(out=stat_in[:, :], in_=st[:])
                nc.gpsimd.collective_compute(
                    "AllReduce", mybir.AluOpType.add,
                    replica_groups=[list(range(NCORES))],
                    ins=[stat_in.ap().opt()], outs=[stat_out.ap().opt()])
                gst = mp.tile([128, 4], F32)
                nc.sync.dma_start(out=gst[:], in_=stat_out[:, :])

                sc = mp.tile([128, 8], F32)
                inv_n = 1.0 / N
                for oc in range(2):
                    mean = sc[:, oc:oc + 1]
                    nc.vector.tensor_scalar_mul(out=mean, in0=gst[:, oc:oc + 1], scalar1=inv_n)
                    ex2 = sc[:, 2 + oc:3 + oc]
                    nc.vector.tensor_scalar_mul(out=ex2, in0=gst[:, 2 + oc:3 + oc], scalar1=inv_n)
                    var = sc[:, 4 + oc:5 + oc]
                    nc.vector.tensor_tensor(out=var, in0=mean, in1=mean, op=mybir.AluOpType.mult)
                    nc.vector.tensor_tensor(out=var, in0=ex2, in1=var, op=mybir.AluOpType.subtract)
                    std = sc[:, 6 + oc:7 + oc]
                    nc.scalar.activation(out=std, in_=var, func=mybir.ActivationFunctionType.Sqrt, bias=epst[:])
                    nc.vector.reciprocal(out=std, in_=std)  # now rstd
                    scale = sc[:, 4 + oc:5 + oc]  # overwrite var slot
                    nc.vector.tensor_tensor(out=scale, in0=gb[:, oc:oc + 1], in1=std, op=mybir.AluOpType.mult)
                    tmp = sc[:, oc:oc + 1]  # mean slot -> mean*scale
                    nc.vector.tensor_tensor(out=tmp, in0=mean, in1=scale, op=mybir.AluOpType.mult)
                    shift = sc[:, 6 + oc:7 + oc]  # overwrite rstd slot (already consumed)
                    nc.vector.tensor_tensor(out=shift, in0=gb[:, 2 + oc:3 + oc], in1=tmp,
                                            op=mybir.AluOpType.subtract)

                # -- fused BN+ReLU on my shard --
                last = li == len(LAYER_CINS) - 1
                if last:
                    CH = SHARD // 4
                    for oc in range(2):
                        for c0 in range(0, SHARD, CH):
                            ho = mp.tile([128, CH], F32, name="ho")
                            nc.scalar.activation(out=ho[:], in_=o_t[oc][:, c0:c0 + CH],
                                                 func=mybir.ActivationFunctionType.Relu,
                                                 scale=sc[:, 4 + oc:5 + oc], bias=sc[:, 6 + oc:7 + oc])
                            nc.sync.dma_start(out=out_ext[oc, :, c0:c0 + CH], in_=ho[:])
                else:
                    for oc in range(2):
                        nc.scalar.activation(out=hnew_bf[oc][:], in_=o_t[oc][:],
                                             func=mybir.ActivationFunctionType.Relu,
                                             scale=sc[:, 4 + oc:5 + oc], bias=sc[:, 6 + oc:7 + oc])
    nc.compile()
    return nc


def _host_prep(x, edge_index, W2_0):
    row, col = edge_index[0].astype(np.int64), edge_index[1].astype(np.int64)

    deg = np.bincount(row, minlength=N)
    order = np.argsort(-deg, kind="stable")          # new global rank -> old id
    newpos = np.empty(N, np.int64)
    for s in range(NCORES):
        olds = order[s::NCORES]
        newpos[olds] = s * SHARD + np.arange(olds.shape[0])

    rnew, cnew = newpos[row], newpos[col]

    # table row of each source under split-AllGather layout, and its window
    s_src, o_src = cnew // SHARD, cnew % SHARD
    in_a = o_src < ROW_A
    trow = np.where(in_a, s_src * ROW_A + o_src,
                    s_src * ROW_B + (o_src - ROW_A))   # already window-local
    win = (~in_a).astype(np.int64)

    # shared slab structure: per (tile, window) max count over cores
    core = rnew // SHARD
    t_loc = (rnew % SHARD) // 128
    p_lane = (rnew % SHARD) % 128
    cnt = np.zeros((NCORES, TILES, 2), np.int64)
    np.add.at(cnt, (core, t_loc, win), 1)
    cmax = cnt.max(axis=0)
    slabs = np.ceil(cmax / 128).astype(np.int64)      # [TILES, 2]

    # adaptive groups with per-window slab cap
    groups_t = []
    cur, a, b = [], 0, 0
    for t in range(TILES):
        w0, w1 = int(slabs[t][0]), int(slabs[t][1])
        if cur and max(a + w0, b + w1) > GROUP_CAP:
            groups_t.append(cur); cur, a, b = [], 0, 0
        cur.append(t); a += w0; b += w1
    groups_t.append(cur)

    # global slab order: per group, window-A slabs of its tiles then window-B
    tsoff = np.zeros((TILES, 2), np.int64)
    groups = []
    off = 0
    for tl in groups_t:
        spans = []
        for w in (0, 1):
            st = off
            for t in tl:
                tsoff[t][w] = off
                off += int(slabs[t][w])
            spans.append((st, off - st))
        groups.append((tl, spans))
    S = off

    # per-core idx / dvec tables
    idx_all = np.zeros((NCORES, 128, S * 8), np.int16)
    dvec_all = np.full((NCORES, 128, S), -1.0, np.float32)
    slot_base = np.zeros((TILES, 2), np.int64)
    slot_base[:, :] = tsoff * 128
    for s in range(NCORES):
        m = core == s
        et, ew, eloc, ep = t_loc[m], win[m], trow[m], p_lane[m]
        key = et * 2 + ew
        srt = np.argsort(key, kind="stable")
        key_s = key[srt]
        kcnt = np.bincount(key_s, minlength=TILES * 2)
        kbase = np.concatenate([[0], np.cumsum(kcnt)])[:-1]
        rank = np.arange(key_s.shape[0]) - kbase[key_s]
        slot = slot_base.reshape(-1)[key_s] + rank
        slab_j, lane = slot // 128, slot % 128
        # idx: lane l of slab j -> (partition l%16 in all 8 groups, col 8j+l//16)
        cols = 8 * slab_j + lane // 16
        prow = (lane % 16).astype(np.int64)
        v16 = eloc[srt].astype(np.int16)
        for g in range(8):
            idx_all[s, g * 16 + prow, cols] = v16
        dvec_all[s, lane, slab_j] = ep[srt].astype(np.float32)

    # layer-0 tables: x @ W2_0 in table order
    xg = np.zeros((NP, x.shape[1]), np.float32)
    for s in range(NCORES):
        olds = order[s::NCORES]
        xg[s * SHARD:s * SHARD + olds.shape[0]] = x[olds]
    t2 = (xg @ np.asarray(W2_0, np.float32)).astype(ml_dtypes.bfloat16)
    t2 = t2.reshape(NCORES, SHARD, C)
    t2A = np.ascontiguousarray(t2[:, :ROW_A, :].reshape(WA_ROWS, C))
    t2B = np.ascontiguousarray(t2[:, ROW_A:, :].reshape(WB_ROWS, C))

    x_me = np.zeros((NCORES, 128, SHARD), ml_dtypes.bfloat16)
    for s in range(NCORES):
        olds = order[s::NCORES]
        x_me[s, :, :olds.shape[0]] = x[olds].T.astype(ml_dtypes.bfloat16)

    return (order, slabs, groups, tsoff, S, idx_all, dvec_all, t2A, t2B, x_me)


def kernel(x, edge_index, W1_0, W2_0, g_0, b_0, W1_1, W2_1, g_1, b_1, W1_2, W2_2, g_2, b_2):
    x = np.asarray(x, np.float32)
    ei = np.asarray(edge_index)

    (order, slabs, groups, tsoff, S, idx_all, dvec_all, t2A, t2B, x_me) = \
        _host_prep(x, ei, W2_0)

    weights = {"W1_0": W1_0, "W2_0": W2_0, "g_0": g_0, "b_0": b_0,
               "W1_1": W1_1, "W2_1": W2_1, "g_1": g_1, "b_1": b_1,
               "W1_2": W1_2, "W2_2": W2_2, "g_2": g_2, "b_2": b_2}
    weights = {k: np.ascontiguousarray(np.asarray(v, np.float32)) for k, v in weights.items()}

    nc = _build_nc([list(map(int, s)) for s in slabs], groups,
                   [list(map(int, s)) for s in tsoff], S)
    in_maps = []
    for s in range(NCORES):
        m = {"x_me": np.ascontiguousarray(x_me[s]),
             "t2l0A": t2A, "t2l0B": t2B,
             "idx": np.ascontiguousarray(idx_all[s]),
             "dvec": np.ascontiguousarray(dvec_all[s])}
        m.update(weights)
        in_maps.append(m)

    _tr = bool(os.environ.get("BASS_TRACE_RUN"))
    _td = os.environ.get("BASS_TRACE_DIR") or None
    if _td:
        os.makedirs(_td, exist_ok=True)
    r = run_bass_kernel_spmd(nc, in_maps, list(range(NCORES)), trace=_tr, tmpdir=_td)
    if _tr:
        print("HW exec time:", r.exec_time_ns, "ns", flush=True)
        if r.profile_json:
            print("profile_json:", r.profile_json, flush=True)
    res = r.results

    out = np.empty((N, C), np.float32)
    for s in range(NCORES):
        o = res[s]["out"] if isinstance(res[s], dict) else res[s][0]
        o = np.asarray(o).reshape(2, 128, SHARD)
        blk = np.transpose(o, (2, 0, 1)).reshape(SHARD, C)
        out[order[s::NCORES]] = blk[:6250]
    return out
